# revision 4
# baseline (speedup 1.0000x reference)
"""GCN encoder (3x GCNConv + residual + final linear) on 8 trn2 NeuronCores.

Strategy (graph/data parallel, dst-node sharding):
  * Nodes are sorted by in-degree and dealt round-robin to the 8 cores, so
    every (core, block-of-128-dsts) pair has a near-identical max in-degree.
    One SPMD program (uniform per-block slot counts) with only a few %
    padding waste.
  * Per layer, each core computes h@W for its own 6250 nodes, scales rows by
    dinv (rsqrt degree), and the 8 shards are AllGathered into a replicated
    HBM table of "hs" rows.
  * Aggregation uses the ANT dma_gather instruction (int16 indices): slots
    are identity-mapped (partition p == dst-within-block), one gather slot
    per in-edge plus one self-loop slot, padding slots point at a
    guaranteed-zero table row.  int16 only addresses 32768 rows, so each
    block gathers twice from two overlapping table windows (rows [0,32768)
    and [17408,50176)); edges whose source falls in the overlap are assigned
    to whichever window balances the per-dst slot counts.
  * K identity matmuls per block accumulate the slots into PSUM (fp32), then
    out = relu(acc*dinv + bias) + h_prev on DVE/ACT.
Pad nodes have dinv=0 so their table rows stay exactly zero in every layer.
"""

import os
import numpy as np

import concourse.bass as bass
import concourse.mybir as mybir
import concourse.tile as tile
import concourse.bacc as bacc
from concourse.bass_utils import run_bass_kernel_spmd
from concourse.masks import make_identity

N = 50000
E = 800000
D_IN = 128
D_H = 128
D_OUT = 64
NCORES = 8
P = 128
PER_CORE = 6272          # ceil(50000/8)=6250 padded to 49*128
NBLK = PER_CORE // P     # 49
ROWS = NCORES * PER_CORE # AllGather table rows (50176)
WSZ = 32768
HI_BASE = ROWS - WSZ     # 17408
PAD_LO = 6250            # core 0 pad row (zero), inside low window
PAD_HI = 3 * PER_CORE + 6250 - HI_BASE  # core 3 pad row rebased (7658)
MAX_GROUP_SLOTS = 168    # chunks per gather group (SBUF budget)

TAB_F32 = os.environ.get("GCN_TAB", "bf16") == "f32"
LAST_EXEC_NS = None
LAST_RESULTS = None
_CACHE = {}


def _preprocess(x, edge_index):
    src = edge_index[0].astype(np.int64)
    dst = edge_index[1].astype(np.int64)
    deg_in = np.bincount(dst, minlength=N)
    dinv = (1.0 / np.sqrt((deg_in + 1).astype(np.float32))).astype(np.float32)

    perm = np.argsort(deg_in, kind="stable")     # rank r -> orig node perm[r]
    rank = np.empty(N, np.int64)
    rank[perm] = np.arange(N)
    trow = (rank % NCORES) * PER_CORE + rank // NCORES

    degs_sorted = deg_in[perm]
    order = np.argsort(rank[dst], kind="stable")
    srcs_t = trow[src[order]]                    # edges grouped by dst rank
    cuts = np.zeros(N + 1, np.int64)
    cuts[1:] = np.cumsum(degs_sorted)

    # per-node slot lists split across the two windows, balanced via overlap
    self_t = trow[perm]                          # self table row per rank
    lo_lists = [None] * N
    hi_lists = [None] * N
    n_lo = np.zeros(N, np.int32)
    n_hi = np.zeros(N, np.int32)
    for r in range(N):
        tl = srcs_t[cuts[r]:cuts[r + 1]]
        tl = np.append(tl, self_t[r])
        fixed_lo = tl[tl < HI_BASE]
        fixed_hi = tl[tl >= WSZ]
        flex = tl[(tl >= HI_BASE) & (tl < WSZ)]
        tot = len(tl)
        want_lo = (tot + 1) // 2
        nl = min(max(want_lo, len(fixed_lo)), tot - len(fixed_hi))
        take = nl - len(fixed_lo)
        lo_lists[r] = np.concatenate([fixed_lo, flex[:take]])
        hi_lists[r] = np.concatenate([fixed_hi, flex[take:]]) - HI_BASE
        n_lo[r] = nl
        n_hi[r] = tot - nl

    # per-block slot counts (uniform across cores by construction)
    Klo = np.zeros(NBLK, np.int64)
    Khi = np.zeros(NBLK, np.int64)
    for b in range(NBLK):
        lo, hi = b * 1024, min((b + 1) * 1024, N)
        Klo[b] = n_lo[lo:hi].max()
        Khi[b] = n_hi[lo:hi].max()

    # pack blocks into gather groups bounded by SBUF slot budget
    groups = []
    cur = []
    cur_slots = 0
    for b in range(NBLK):
        s = int(Klo[b] + Khi[b])
        if cur and cur_slots + s > MAX_GROUP_SLOTS:
            groups.append(cur)
            cur = []
            cur_slots = 0
        cur.append(b)
        cur_slots += s
    groups.append(cur)

    # fill slot value arrays [core][p, columns] per side
    lo_off = np.zeros(NBLK, np.int64)
    lo_off[1:] = np.cumsum(Klo)[:-1]
    hi_off = np.zeros(NBLK, np.int64)
    hi_off[1:] = np.cumsum(Khi)[:-1]
    SKlo, SKhi = int(Klo.sum()), int(Khi.sum())
    vlo = np.full((NCORES, P, SKlo), PAD_LO, np.int32)
    vhi = np.full((NCORES, P, SKhi), PAD_HI, np.int32)
    for r in range(N):
        c = r % NCORES
        pos = r // NCORES
        b, p = pos // P, pos % P
        ll, hl = lo_lists[r], hi_lists[r]
        vlo[c, p, lo_off[b]:lo_off[b] + len(ll)] = ll
        vhi[c, p, hi_off[b]:hi_off[b] + len(hl)] = hl

    # wrapped int16 index tensors, one column range per (group, side)
    def wrap16(flat):
        # flat[i] must land at dest slot (p=i%128, chunk=i//128);
        # ucode reads index i from [i%16, i//16], replicated over 8 Q7 stripes
        w = flat.astype(np.int16).reshape(-1, 16).T
        return np.tile(w, (8, 1))

    per_core_idx16 = []
    meta = []          # (side, col_off, ncols, num_idxs, group blocks)
    col = 0
    for g in groups:
        for side, v_off, Ks in (("lo", lo_off, Klo), ("hi", hi_off, Khi)):
            ni = int(sum(Ks[b] for b in g)) * P
            meta.append((side, col, ni // 16, ni, g))
            col += ni // 16
    for c in range(NCORES):
        arr = np.empty((P, col), np.int16)
        cc = 0
        for g in groups:
            for side, v, Ks, off in (("lo", vlo, Klo, lo_off), ("hi", vhi, Khi, hi_off)):
                cols = np.concatenate(
                    [v[c][:, off[b]:off[b] + Ks[b]] for b in g], axis=1)
                ni = cols.shape[1] * P
                arr[:, cc:cc + ni // 16] = wrap16(cols.T.ravel())
                cc += ni // 16
        per_core_idx16.append(arr)

    x_sh = np.zeros((NCORES, PER_CORE, D_IN), np.float32)
    dinv_sh = np.zeros((NCORES, P, NBLK), np.float32)
    for c in range(NCORES):
        r = np.arange(PER_CORE, dtype=np.int64) * NCORES + c
        valid = r < N
        x_sh[c, valid] = x[perm[r[valid]]]
        dv = np.zeros(PER_CORE, np.float32)
        dv[valid] = dinv[perm[r[valid]]]
        dinv_sh[c] = dv.reshape(NBLK, P).T

    plan = dict(Klo=Klo, Khi=Khi, groups=groups, meta=meta, ncols16=col)
    return plan, per_core_idx16, x_sh, dinv_sh, perm


def _build(plan):
    f32 = mybir.dt.float32
    i16 = mybir.dt.int16
    tabdt = f32 if TAB_F32 else mybir.dt.bfloat16
    Klo, Khi = plan["Klo"], plan["Khi"]
    groups, meta = plan["groups"], plan["meta"]
    ncols16 = plan["ncols16"]
    max_slots = max(int(sum(Klo[b] + Khi[b] for b in g)) for g in groups)

    nc = bacc.Bacc("TRN2", target_bir_lowering=False, debug=False,
                   enable_asserts=True, num_devices=NCORES)

    x_t = nc.dram_tensor("x_sh", [PER_CORE, D_IN], f32, kind="ExternalInput")
    w_t = [nc.dram_tensor(f"w{i}", [D_H, D_H], f32, kind="ExternalInput") for i in range(3)]
    b_t = [nc.dram_tensor(f"b{i}", [P, D_H], f32, kind="ExternalInput") for i in range(3)]
    wout_t = nc.dram_tensor("wout", [D_H, D_OUT], f32, kind="ExternalInput")
    bout_t = nc.dram_tensor("bout", [P, D_OUT], f32, kind="ExternalInput")
    dinv_t = nc.dram_tensor("dinv_sh", [P, NBLK], f32, kind="ExternalInput")
    idx_t = nc.dram_tensor("idx16", [P, ncols16], i16, kind="ExternalInput")
    y_t = nc.dram_tensor("y_sh", [PER_CORE, D_OUT], f32, kind="ExternalOutput")

    rg = [list(range(NCORES))]
    add = mybir.AluOpType.add
    relu = mybir.ActivationFunctionType.Relu

    with tile.TileContext(nc) as tc:
        with (
            tc.tile_pool(name="const", bufs=1) as cpool,
            tc.tile_pool(name="work", bufs=3) as wpool,
            tc.tile_pool(name="gbuf", bufs=2) as gpool,
            tc.tile_pool(name="hbuf", bufs=2) as hpool,
            tc.tile_pool(name="psum", bufs=2, space="PSUM") as ppool,
            tc.tile_pool(name="dram", bufs=2, space="DRAM") as dpool,
        ):
            ident = cpool.tile([P, P], f32)
            make_identity(nc, ident[:])
            if TAB_F32:
                ident_g = ident
            else:
                ident_g = cpool.tile([P, P], tabdt)
                make_identity(nc, ident_g[:])

            wt, bt = [], []
            for i in range(3):
                w_s = cpool.tile([D_H, D_H], f32, name=f"w_s{i}")
                nc.sync.dma_start(out=w_s[:], in_=w_t[i][:])
                b_s = cpool.tile([P, D_H], f32, name=f"b_s{i}")
                nc.sync.dma_start(out=b_s[:], in_=b_t[i][:])
                wt.append(w_s)
                bt.append(b_s)
            wout_s = cpool.tile([D_H, D_OUT], f32)
            nc.sync.dma_start(out=wout_s[:], in_=wout_t[:])
            bout_s = cpool.tile([P, D_OUT], f32)
            nc.sync.dma_start(out=bout_s[:], in_=bout_t[:])
            dinv_s = cpool.tile([P, NBLK], f32)
            nc.sync.dma_start(out=dinv_s[:], in_=dinv_t[:])
            idx_s = cpool.tile([P, ncols16], i16)
            nc.sync.dma_start(out=idx_s[:], in_=idx_t[:])

            h = hpool.tile([P, NBLK * P], f32, tag="h", name="h0")
            for b in range(NBLK):
                nc.sync.dma_start(out=h[:, b * P:(b + 1) * P],
                                  in_=x_t[b * P:(b + 1) * P, :])

            mi = 0
            for layer in range(3):
                ag_in = dpool.tile([PER_CORE, D_H], tabdt, tag="ag_in",
                                   name=f"ag_in{layer}")
                table = dpool.tile([ROWS, D_H], tabdt, tag="table",
                                   addr_space="Shared", name=f"table{layer}")
                hn = hpool.tile([P, NBLK * P], f32, tag="h", name=f"h{layer + 1}")

                for b in range(NBLK):
                    hsl = h[:, b * P:(b + 1) * P]
                    tp = ppool.tile([P, P], f32, tag="tp", name="tp")
                    nc.tensor.transpose(tp[:], hsl, ident[:])
                    hT = wpool.tile([P, P], f32, tag="hT", name="hT")
                    nc.any.tensor_copy(hT[:], tp[:])
                    hw = ppool.tile([P, P], f32, tag="hw", name="hw")
                    nc.tensor.matmul(hw[:], lhsT=hT[:], rhs=wt[layer][:],
                                     start=True, stop=True)
                    hs = wpool.tile([P, D_H], tabdt, tag="hs", name="hs")
                    nc.vector.tensor_scalar_mul(hs[:], hw[:], dinv_s[:, b:b + 1])
                    nc.sync.dma_start(out=ag_in[b * P:(b + 1) * P, :], in_=hs[:])

                nc.gpsimd.collective_compute(
                    "AllGather", mybir.AluOpType.bypass, replica_groups=rg,
                    ins=[ag_in[:]], outs=[table[:]],
                )

                gi = 0
                for g in groups:
                    nlo = int(sum(Klo[b] for b in g))
                    nhi = int(sum(Khi[b] for b in g))
                    G = gpool.tile([P, max_slots, P], tabdt, tag="G", name="G")
                    for (side, col, ncols, ni, _g), sl in (
                        (meta[gi], slice(0, nlo)),
                        (meta[gi + 1], slice(nlo, nlo + nhi)),
                    ):
                        win = table[0:WSZ, :] if side == "lo" else table[HI_BASE:ROWS, :]
                        nc.gpsimd.dma_gather(
                            out_ap=G[:, sl, :], in_ap=win,
                            idxs_ap=idx_s[:, col:col + ncols],
                            num_idxs=ni, num_idxs_reg=ni, elem_size=P,
                            single_packet=False,
                        )
                    gi += 2
                    lo_c = 0
                    hi_c = nlo
                    for b in g:
                        kl, kh = int(Klo[b]), int(Khi[b])
                        chunks = list(range(lo_c, lo_c + kl)) + \
                                 list(range(hi_c, hi_c + kh))
                        lo_c += kl
                        hi_c += kh
                        acc = ppool.tile([P, P], f32, tag="acc", name="acc")
                        for j, ch in enumerate(chunks):
                            nc.tensor.matmul(acc[:], lhsT=ident_g[:],
                                             rhs=G[:, ch, :],
                                             start=(j == 0),
                                             stop=(j == len(chunks) - 1))
                        t = wpool.tile([P, P], f32, tag="t", name="t")
                        nc.vector.tensor_scalar_mul(t[:], acc[:], dinv_s[:, b:b + 1])
                        nc.vector.tensor_tensor(out=t[:], in0=t[:],
                                                in1=bt[layer][:], op=add)
                        hns = hn[:, b * P:(b + 1) * P]
                        nc.scalar.activation(hns, t[:], relu)
                        nc.vector.tensor_tensor(out=hns, in0=hns,
                                                in1=h[:, b * P:(b + 1) * P], op=add)
                h = hn
                mi += 0

            for b in range(NBLK):
                hsl = h[:, b * P:(b + 1) * P]
                tp = ppool.tile([P, P], f32, tag="tp", name="tpf")
                nc.tensor.transpose(tp[:], hsl, ident[:])
                hT = wpool.tile([P, P], f32, tag="hT", name="hTf")
                nc.any.tensor_copy(hT[:], tp[:])
                yp = ppool.tile([P, D_OUT], f32, tag="acc", name="yp")
                nc.tensor.matmul(yp[:], lhsT=hT[:], rhs=wout_s[:],
                                 start=True, stop=True)
                yt = wpool.tile([P, D_OUT], f32, tag="t", name="yt")
                nc.vector.tensor_tensor(out=yt[:], in0=yp[:], in1=bout_s[:], op=add)
                nc.sync.dma_start(out=y_t[b * P:(b + 1) * P, :], in_=yt[:])

    nc.compile()
    return nc


def kernel(x, edge_index, W0, b0, W1, b1, W2, b2, W_out, b_out):
    global LAST_EXEC_NS, LAST_RESULTS
    x = np.asarray(x, dtype=np.float32)
    edge_index = np.asarray(edge_index, dtype=np.int32)
    Ws = [np.asarray(w, np.float32) for w in (W0, W1, W2)]
    bs = [np.asarray(b, np.float32) for b in (b0, b1, b2)]
    W_out = np.asarray(W_out, np.float32)
    b_out = np.asarray(b_out, np.float32)

    plan, idx16, x_sh, dinv_sh, perm = _preprocess(x, edge_index)
    nc = _build(plan)

    in_maps = []
    for c in range(NCORES):
        m = {
            "x_sh": np.ascontiguousarray(x_sh[c]),
            "dinv_sh": np.ascontiguousarray(dinv_sh[c]),
            "idx16": np.ascontiguousarray(idx16[c]),
            "wout": W_out,
            "bout": np.ascontiguousarray(np.broadcast_to(b_out[None, :], (P, D_OUT))),
        }
        for i in range(3):
            m[f"w{i}"] = Ws[i]
            m[f"b{i}"] = np.ascontiguousarray(np.broadcast_to(bs[i][None, :], (P, D_H)))
        in_maps.append(m)

    trace = os.environ.get("GCN_TRACE", "0") == "1"
    res = run_bass_kernel_spmd(nc, in_maps, list(range(NCORES)), trace=trace)
    LAST_EXEC_NS = res.exec_time_ns
    LAST_RESULTS = res

    y = np.empty((N, D_OUT), np.float32)
    for c in range(NCORES):
        r = np.arange(PER_CORE, dtype=np.int64) * NCORES + c
        valid = r < N
        y[perm[r[valid]]] = res.results[c]["y_sh"][valid]
    return y
